# revision 15
# baseline (speedup 1.0000x reference)
"""Trainium2 Bass kernel for nn_ComplexDotProduct.

  out[b, o, n] = sum_c complex(x)[b, c, n] * complex(w)[o, c, n] + bias[o]
  B=64, C=128, N=1024, O=512.

Strategy
--------
Shard N across the 8 cores (128 positions each) — no tensor is replicated,
so per-core HBM traffic is the global minimum. All tensors move as bf16
(rel err ~3e-3, gate is 2e-2), halving traffic vs fp32: per core
w 33.5 MB + x 6.3 MB + out 16.8 MB ~= 57 MB -> ~158 us at the 358 GB/s
per-core HBM roofline (the kernel is DMA-bound at ~330 GB/s effective;
compute is ~55 us and fully hidden). DMA rings: w split across the two
HWDGE rings (SP + ACT), x on ACT, store on the gpsimd SWDGE ring so store
descriptors never queue behind w-loads. Tile-pool bufs divide the 16
j-tiles so the hardware-loop boundary has no zero-gap buffer WAR.

Complex packing: per position x is packed as 192 columns [-im | re | im]
(64 each). Two overlapping 128-column windows give the two stationary
operands S2 = [-im | re] (cols 0:128) and S1 = [re | im] (cols 64:192).
Accumulating
  PSUM  = S1^T @ w_re   (partitions 0-63: re*w_re,   64-127: im*w_re)
  PSUM += S2^T @ w_im   (partitions 0-63: -im*w_im,  64-127: re*w_im)
yields partitions 0-63 = Re(out), 64-127 = Im(out) for one position in a
single PSUM bank — 2 full-width (128-col stationary) matmuls per position
instead of 4 half-width ones, and a full 128-partition output tile (all
16 SDMA engines reachable on the store, no shuffle needed).

Bias (per o, re/im by partition half) is fused into the DVE PSUM->SBUF
evacuation, which also casts to bf16.

Host-side prep packs x as (C, N, 192) [-im|re|im] and w as (C, N, 2*O)
[re|im] bf16 so every DMA is long-contiguous per partition; the kernel
writes out as (128, NSH, O) bf16 per core and the host assembles
complex64 (B, O, N).
"""

import numpy as np

B, C, N, O = 64, 128, 1024, 512
NCORES = 8
NSH = N // NCORES        # 128 positions per core
JT = 8                   # positions per j-tile
NT = NSH // JT           # j-tiles per core
XCOLS = 3 * B            # [-im | re | im]


def build_nc(loop_r=None, timing_pool=None, parts="all", jt=None, bufs=(8, 4),
             split_w=True, store_q="gpsimd", x128=None):
    """Build the per-core Tile program.

    loop_r: wrap the body in a hardware For_i loop (timing only).
    timing_pool: if set (e.g. 2), DRAM in/out tensors cover only that many
    j-tiles and the body cycles through them — keeps the uploaded bytes tiny
    for loop-delta timing while preserving per-iteration DMA/compute work.
    parts: "all" | "dma" (skip compute) | "noout" (skip output store).
    split_w: issue the w load as two halves on the SP and ACT HWDGE rings.
    store_q: "sync" | "scalar" | "gpsimd" — queue for the output store.
    x128: ship x as 128 DRAM cols [im|re] (position-innermost layout) and
    build the -im block on DVE, instead of 192 DRAM cols [-im|re|im].
    With x128 the PSUM halves swap: partitions 0-63 = Im, 64-127 = Re.
    """
    import concourse.mybir as mybir
    from concourse import bacc
    from concourse.tile import TileContext

    bf16 = mybir.dt.bfloat16
    f32 = mybir.dt.float32
    add = mybir.AluOpType.add

    nc = bacc.Bacc(None, target_bir_lowering=False, debug=False)

    x128 = X128 if x128 is None else x128
    jt = JT if jt is None else jt
    nt = NSH // jt
    pool_n = NSH if timing_pool is None else timing_pool * jt
    if x128:
        x_d = nc.dram_tensor("xt", (C, pool_n // jt, 2 * B, jt), bf16,
                             kind="ExternalInput")
    else:
        x_d = nc.dram_tensor("xt", (C, pool_n, XCOLS), bf16,
                             kind="ExternalInput")
    w_d = nc.dram_tensor("wt", (C, pool_n, 2 * O), bf16, kind="ExternalInput")
    b_d = nc.dram_tensor("bt", (2 * B, O), f32, kind="ExternalInput")
    out_d = nc.dram_tensor("out", (2 * B, pool_n, O), bf16,
                           kind="ExternalOutput")

    with TileContext(nc) as tc:
        with (
            tc.tile_pool(name="xw", bufs=bufs[0]) as xw,
            tc.tile_pool(name="ob", bufs=bufs[1]) as ob,
            tc.tile_pool(name="cst", bufs=1) as cst,
            tc.tile_pool(name="ps", bufs=4, space="PSUM") as ps,
        ):
            b_t = cst.tile([2 * B, O], f32)
            nc.sync.dma_start(out=b_t[:], in_=b_d[:])

            def one_position(x_t, w_t, o_t, j):
                ps_t = ps.tile([2 * B, O], mybir.dt.float32, name="ps")
                if x128:
                    # x_t is [C, 192, jt] = [im|re|-im]; S_a = [im|re],
                    # S_b = [re|-im] -> partitions 0-63 Im, 64-127 Re
                    s_a = x_t[:, 0:2 * B, j]
                    s_b = x_t[:, B:XCOLS, j]
                else:
                    # x_t is [C, jt, 192] = [-im|re|im]; S_a = [re|im],
                    # S_b = [-im|re] -> partitions 0-63 Re, 64-127 Im
                    s_a = x_t[:, j, B:XCOLS]
                    s_b = x_t[:, j, 0:2 * B]
                nc.tensor.matmul(ps_t[:], s_a, w_t[:, j, 0:O],
                                 start=True, stop=False)
                nc.tensor.matmul(ps_t[:], s_b, w_t[:, j, O:2 * O],
                                 start=False, stop=True)
                nc.vector.tensor_tensor(o_t[:, j, :], ps_t[:], b_t[:], add)

            store_eng = {"sync": nc.sync, "scalar": nc.scalar,
                         "gpsimd": nc.gpsimd}[store_q]

            def body(_i=None):
                for jt_i in range(nt):
                    if x128:
                        x_t = xw.tile([C, XCOLS, jt], bf16, name="x_t")
                    else:
                        x_t = xw.tile([C, jt, XCOLS], bf16, name="x_t")
                    w_t = xw.tile([C, jt, 2 * O], bf16, name="w_t")
                    o_t = ob.tile([2 * B, jt, O], bf16, name="o_t")
                    eff = jt_i if timing_pool is None else jt_i % timing_pool
                    sl = slice(eff * jt, (eff + 1) * jt)
                    if x128:
                        nc.scalar.dma_start(out=x_t[:, 0:2 * B, :],
                                            in_=x_d[:, eff])
                        if parts != "dma":
                            nc.vector.tensor_scalar_mul(
                                x_t[:, 2 * B:XCOLS, :], x_t[:, 0:B, :], -1.0)
                    else:
                        nc.scalar.dma_start(out=x_t[:], in_=x_d[:, sl])
                    if split_w:
                        h = jt // 2
                        nc.sync.dma_start(out=w_t[:, :h], in_=w_d[:, sl][:, :h])
                        nc.scalar.dma_start(out=w_t[:, h:], in_=w_d[:, sl][:, h:])
                    else:
                        nc.sync.dma_start(out=w_t[:], in_=w_d[:, sl])
                    for j in range(jt) if parts != "dma" else []:
                        one_position(x_t, w_t, o_t, j)
                    if parts != "noout":
                        if parts == "dma":
                            nc.vector.memset(o_t[0:1, 0, 0:1], 0.0)
                        store_eng.dma_start(out=out_d[:, sl], in_=o_t[:])

            if loop_r is None:
                body()
            else:
                with tc.For_i(0, loop_r, 1):
                    body()

    nc.compile()
    return nc


X128 = False  # x-packing choice; the x128=True path measured slower (see doc)


def _prep_inputs(x_re, x_im, w_re, w_im, b_re, b_im, x128=X128, jt=JT):
    """Host-side packing/transposition into the kernel's DMA-friendly bf16
    layouts. Threaded over blocks to speed up the big w transpose."""
    from concurrent.futures import ThreadPoolExecutor
    import ml_dtypes

    bf16 = ml_dtypes.bfloat16
    x_re = np.asarray(x_re, dtype=np.float32)
    x_im = np.asarray(x_im, dtype=np.float32)
    w_re = np.asarray(w_re, dtype=np.float32)
    w_im = np.asarray(w_im, dtype=np.float32)
    b_re = np.asarray(b_re, dtype=np.float32)
    b_im = np.asarray(b_im, dtype=np.float32)

    xcols = 2 * B if x128 else XCOLS
    xt = np.empty((C, N, xcols), bf16)
    # wt: (C, N, 2*O) <- [w_re | w_im] transposed from (O, C, N)
    wt = np.empty((C, N, 2 * O), bf16)

    def do_x(k):
        if x128:
            # xt: (C, N, 128) <- [x_im | x_re]
            if k == 0:
                xt[:, :, B:] = x_re.transpose(1, 2, 0)
            else:
                xt[:, :, :B] = x_im.transpose(1, 2, 0)
        else:
            # xt: (C, N, 192) <- [-x_im | x_re | x_im]
            if k == 0:
                xt[:, :, B:2 * B] = x_re.transpose(1, 2, 0)
            else:
                im = x_im.transpose(1, 2, 0)
                xt[:, :, 2 * B:] = im
                xt[:, :, :B] = -im

    def do_w(args):
        k, c0 = args
        src = w_re[0] if k == 0 else w_im[0]
        # copy block of c rows: dst (cblk, N, O) <- src (O, cblk, N)
        wt[c0:c0 + 16, :, k * O:(k + 1) * O] = \
            src[:, c0:c0 + 16, :].transpose(1, 2, 0)

    with ThreadPoolExecutor(max_workers=16) as ex:
        futs = [ex.submit(do_x, k) for k in range(2)]
        futs += [ex.submit(do_w, (k, c0)) for k in range(2)
                 for c0 in range(0, C, 16)]
        for f in futs:
            f.result()

    bt = np.empty((2 * B, O), np.float32)
    lo, hi = (b_im, b_re) if x128 else (b_re, b_im)
    bt[:B, :] = lo[0, :, 0][None, :]
    bt[B:, :] = hi[0, :, 0][None, :]

    in_maps = []
    for c in range(NCORES):
        sl = slice(c * NSH, (c + 1) * NSH)
        xc = xt[:, sl]
        if x128:
            # (C, NSH, 128) -> (C, NT, 128, jt) position-innermost
            xc = np.ascontiguousarray(
                xc.reshape(C, NSH // jt, jt, 2 * B).transpose(0, 1, 3, 2))
        else:
            xc = np.ascontiguousarray(xc)
        in_maps.append({
            "xt": xc,
            "wt": np.ascontiguousarray(wt[:, sl]),
            "bt": bt,
        })
    return in_maps


def _assemble(results, x128=X128):
    """Per-core 'out' buffers (128, NSH, O) bf16 -> (B, O, N) complex64."""
    from concurrent.futures import ThreadPoolExecutor

    out = np.empty((B, O, N), np.complex64)

    def do_core(c):
        buf = results[c]["out"]
        lo = np.asarray(buf[:B], np.float32)      # (B, NSH, O)
        hi = np.asarray(buf[B:], np.float32)
        re, im = (hi, lo) if x128 else (lo, hi)
        out[:, :, c * NSH:(c + 1) * NSH] = (re + 1j * im).transpose(0, 2, 1)

    with ThreadPoolExecutor(max_workers=NCORES) as ex:
        list(ex.map(do_core, range(NCORES)))
    return out


def kernel(x_re, x_im, w_re, w_im, b_re, b_im):
    from concourse import bass_utils

    nc = build_nc(x128=X128)
    in_maps = _prep_inputs(x_re, x_im, w_re, w_im, b_re, b_im)
    res = bass_utils.run_bass_kernel_spmd(nc, in_maps, core_ids=list(range(NCORES)))
    return _assemble(res.results)
